# revision 1
# baseline (speedup 1.0000x reference)
"""Trainium2 Bass kernel for nn_CompressedCausalAttention.

Sharding: 8 cores = 2 batches x 4 head-groups (2 heads each).
Per-core dataflow (all chan-major "T" layouts are (channel partition, seq free)):
  phase 1: xpeT = (x+pe)^T per c-chunk, bf16
  phase 2: qT,kT chan-major (+bias via ACT); v seq-major with ones column
  phase 3: flash attention, scores in (t-part, s-free) layout, no max subtraction
           (scores bounded; masked entries get -30000 -> exp == 0), softmax
           denominator via the ones column of V through the AV matmul
  phase 4: partial output projection outpT = Wc_mine^T-slice @ attnTn
Host: shards inputs (incl. transposes), sums the 4 per-batch partials, adds bc.
"""

import numpy as np
import ml_dtypes

S, B, C, H = 2048, 2, 512, 8
CC = C // H            # 64
HPC = 2                # heads per core
CPC = HPC * CC         # 128 channels per core
NCORE = 8
SW = 512               # s window (free dim of score tiles)
TCH = 128              # t chunk (partition dim of score tiles)
TEMP = 1.0 / 8.0       # 1/sqrt(CC)
BIGNEG = -30000.0

_CACHE = {}


def _build_bass():
    import concourse.bass as bass
    import concourse.mybir as mybir
    import concourse.tile as tile
    from concourse import bacc

    f32 = mybir.dt.float32
    bf16 = mybir.dt.bfloat16

    nc = bacc.Bacc("TRN2", target_bir_lowering=False)
    xt = nc.declare_dram_parameter("xt", [4, 4, 128, SW], f32, isOutput=False)
    pet = nc.declare_dram_parameter("pet", [4, 4, 128, SW], f32, isOutput=False)
    w3t = nc.declare_dram_parameter("w3t", [4, 128, 512], bf16, isOutput=False)
    b3 = nc.declare_dram_parameter("b3", [128, 2], f32, isOutput=False)
    wct = nc.declare_dram_parameter("wct", [128, C], bf16, isOutput=False)
    tri = nc.declare_dram_parameter("tri", [128, 128], bf16, isOutput=False)
    id128 = nc.declare_dram_parameter("id128", [128, 128], bf16, isOutput=False)
    outp = nc.declare_dram_parameter("outp", [C, S], bf16, isOutput=True)

    Ident = mybir.ActivationFunctionType.Identity
    Exp = mybir.ActivationFunctionType.Exp

    with tile.TileContext(nc) as tc:
        with (
            tc.tile_pool(name="singles", bufs=1) as singles,
            tc.tile_pool(name="lp", bufs=3) as lp,
            tc.tile_pool(name="pbp", bufs=6) as pbp,
            tc.tile_pool(name="atp", bufs=3) as atp,
            tc.tile_pool(name="rbp", bufs=2) as rbp,
            tc.tile_pool(name="osp", bufs=4) as osp,
            tc.tile_pool(name="psA", bufs=3, space="PSUM") as psA,
            tc.tile_pool(name="psB", bufs=1, space="PSUM") as psB,
        ):
            # ---- constants ----
            tri_sb = singles.tile([128, 128], bf16, tag="tri")
            nc.sync.dma_start(out=tri_sb, in_=tri[:, :])
            id_sb = singles.tile([128, 128], bf16, tag="id128")
            nc.sync.dma_start(out=id_sb, in_=id128[:, :])
            w3t_sb = singles.tile([128, 4, 512], bf16, tag="w3t")
            for k in range(4):
                nc.sync.dma_start(out=w3t_sb[:, k, :], in_=w3t[k])
            b3_sb = singles.tile([128, 2], f32, tag="b3")
            nc.sync.dma_start(out=b3_sb, in_=b3[:, :])
            wct_sb = singles.tile([128, C], bf16, tag="wct")
            nc.sync.dma_start(out=wct_sb, in_=wct[:, :])
            ones1 = singles.tile([1, CC], f32, tag="ones1")
            nc.vector.memset(ones1, 1.0)

            # ---- phase 1+2 (s-blocked): load x/pe, xpeT, qT/kT/v per s-window ----
            xpeT = [singles.tile([128, S], bf16, tag=f"xpeT{k}", name=f"xpeT{k}") for k in range(4)]
            qT = singles.tile([128, S], bf16, tag="qT")
            kT = singles.tile([128, S], bf16, tag="kT")
            # v padded to 128 cols (col CC = ones for the softmax denominator,
            # cols CC+1.. zero) so the AV matmul qualifies for fast weight load
            vsb = singles.tile([128, 16, 2, 128], bf16, tag="vsb")
            nc.vector.memset(vsb[:, :, :, CC:], 0.0)
            nc.vector.memset(vsb[:, :, :, CC:CC + 1], 1.0)
            for w in range(4):
                sl = slice(w * SW, (w + 1) * SW)
                for k in range(4):
                    xtile = lp.tile([128, SW], f32, tag="xtile")
                    petile = lp.tile([128, SW], f32, tag="petile")
                    nc.sync.dma_start(out=xtile, in_=xt[w, k])
                    nc.sync.dma_start(out=petile, in_=pet[w, k])
                    nc.vector.tensor_add(out=xpeT[k][:, sl], in0=xtile, in1=petile)
                for blk, dst in ((0, qT), (1, kT)):
                    ps = psA.tile([128, SW], f32, tag="big")
                    for k in range(4):
                        nc.tensor.matmul(
                            ps,
                            lhsT=w3t_sb[:, k, blk * 128:(blk + 1) * 128],
                            rhs=xpeT[k][:, sl],
                            start=(k == 0),
                            stop=(k == 3),
                        )
                    nc.scalar.activation(
                        out=dst[:, sl], in_=ps, func=Ident,
                        bias=b3_sb[:, blk:blk + 1], scale=1.0,
                    )
                for st0 in (4 * w, 4 * w + 2):
                    ps = psA.tile([128, 2, 2, 128], f32, tag="big")
                    for p_ in range(2):
                        for k in range(4):
                            nc.tensor.matmul(
                                ps[:, p_],
                                lhsT=xpeT[k][:, (st0 + p_) * 128:(st0 + p_ + 1) * 128],
                                rhs=w3t_sb[:, k, 256:512],
                                start=(k == 0),
                                stop=(k == 3),
                            )
                    nc.vector.tensor_copy(
                        out=vsb[:, st0:st0 + 2, :, 0:CC],
                        in_=ps[:, :, :, 0:CC],
                    )

            # ---- phase 3: attention (heads interleaved for PE/ACT overlap) ----
            for i in range(4):
                jmax = 4 * i + 3
                avs = [psB.tile([128, SW], f32, tag=f"av{h}", name=f"av{h}_{i}") for h in range(HPC)]
                atn = atp.tile([128, SW], bf16, tag="atn")
                for j in range(jmax + 1):
                    # D = first unmasked column of this (t-chunk, s-window) pair;
                    # columns [0:D) are strictly-future for every row -> skipped.
                    D = max(0, 128 * j - 512 * i)
                    sc2 = psA.tile([128, 2, SW], f32, tag="big")
                    for h in range(HPC):
                        nc.tensor.matmul(
                            sc2[:, h, D:SW],
                            lhsT=kT[h * CC:(h + 1) * CC, j * TCH:(j + 1) * TCH],
                            rhs=qT[h * CC:(h + 1) * CC, i * SW + D:(i + 1) * SW],
                            start=True, stop=True,
                        )
                        if j >= 4 * i:
                            # staircase mask: accumulate -30000*lower_tri via PE
                            nc.tensor.matmul(
                                sc2[:, h, D:D + 128], lhsT=tri_sb, rhs=id_sb,
                                start=False, stop=True, skip_group_check=True,
                            )
                    pb2 = pbp.tile([128, 2, SW], bf16, tag="pb")
                    nc.scalar.activation(out=pb2[:, :, D:SW], in_=sc2[:, :, D:SW], func=Exp, scale=TEMP)
                    for h in range(HPC):
                        nc.tensor.matmul(
                            avs[h][:, D:SW], lhsT=vsb[:, j, h, :], rhs=pb2[:, h, D:SW],
                            start=(j == 0), stop=(j == jmax),
                        )
                for h in range(HPC):
                    den1 = rbp.tile([1, SW], f32, tag="den1")
                    nc.scalar.copy(out=den1, in_=avs[h][CC:CC + 1, :])
                    bc_ps = psA.tile([CC, SW], f32, tag="big")
                    nc.tensor.matmul(
                        bc_ps, lhsT=ones1, rhs=den1,
                        start=True, stop=True,
                    )
                    rcb = rbp.tile([CC, SW], f32, tag="rcb")
                    nc.vector.reciprocal(out=rcb, in_=bc_ps)
                    nc.vector.tensor_mul(
                        out=atn[h * CC:(h + 1) * CC, :],
                        in0=avs[h][0:CC, :], in1=rcb,
                    )
                # ---- phase 4: partial out-projection for this window ----
                for d in range(4):
                    op = psA.tile([128, SW], f32, tag="big")
                    nc.tensor.matmul(
                        op, lhsT=wct_sb[:, d * 128:(d + 1) * 128],
                        rhs=atn,
                        start=True, stop=True,
                    )
                    ob = osp.tile([128, SW], bf16, tag="ob")
                    nc.scalar.copy(out=ob, in_=op)
                    nc.sync.dma_start(
                        out=outp[d * 128:(d + 1) * 128, i * SW:(i + 1) * SW], in_=ob
                    )

    nc.compile()
    return nc


def _get_nc():
    if "nc" not in _CACHE:
        _CACHE["nc"] = _build_bass()
    return _CACHE["nc"]


def _make_in_maps(x, pe, Wqkv, bqkv, Wc):
    bf = ml_dtypes.bfloat16
    tt = np.arange(128)[:, None]
    kk = np.arange(128)[None, :]
    tri = np.where(kk < tt, np.float32(BIGNEG), np.float32(0.0)).astype(bf).T.copy()
    id128 = np.eye(128, dtype=np.float32).astype(bf)

    xt_b = {}
    pet_b = {}
    for b in range(B):
        t = x[:, b, :].T.reshape(4, 128, 4, SW)
        xt_b[b] = np.ascontiguousarray(t.transpose(2, 0, 1, 3))
        t = pe[:, b, :].T.reshape(4, 128, 4, SW)
        pet_b[b] = np.ascontiguousarray(t.transpose(2, 0, 1, 3))

    in_maps = []
    for core in range(NCORE):
        b, hg = core // 4, core % 4
        lo = hg * 128
        Wv = Wqkv[2 * C + lo:2 * C + lo + 128]
        Vpad = np.zeros((256, C), np.float32)
        Vpad[0:64] = Wv[0:64]
        Vpad[128:192] = Wv[64:128]
        W3 = np.concatenate([Wqkv[lo:lo + 128], Wqkv[C + lo:C + lo + 128], Vpad])
        w3t = np.ascontiguousarray(W3.T).reshape(4, 128, 512).astype(bf)
        b3 = np.stack([bqkv[lo:lo + 128], bqkv[C + lo:C + lo + 128]], axis=1)
        b3 = np.ascontiguousarray(b3).astype(np.float32)
        wct = np.ascontiguousarray(Wc[:, lo:lo + 128].T).astype(bf)
        in_maps.append({
            "xt": xt_b[b], "pet": pet_b[b], "w3t": w3t, "b3": b3,
            "wct": wct, "tri": tri, "id128": id128,
        })
    return in_maps


def _numpy_fallback(x, pe, content_mask, Wqkv, bqkv, Wc, bc):
    xpe = (x + pe).astype(np.float32)
    qkv = xpe.reshape(-1, C) @ Wqkv.T + bqkv
    qkv = qkv.reshape(S, B, 3 * C)
    q, k, v = np.split(qkv, 3, axis=-1)
    q = q.reshape(S, B, H, CC)
    k = k.reshape(S, B, H, CC)
    v = v.reshape(S, B, H, CC)
    out = np.empty((S, B, C), np.float32)
    for b in range(B):
        for h in range(H):
            sc = (q[:, b, h] @ k[:, b, h].T) * np.float32(TEMP)
            sc = np.where(content_mask[:, :, b], -np.inf, sc)
            sc = sc - sc.max(axis=1, keepdims=True)
            p = np.exp(sc)
            p /= p.sum(axis=1, keepdims=True)
            out[:, b, h * CC:(h + 1) * CC] = p @ v[:, b, h]
    return (out.reshape(-1, C) @ Wc.T + bc).reshape(S, B, C).astype(np.float32)


def kernel(x, pe, content_mask, pad, Wqkv, bqkv, Wc, bc):
    x = np.asarray(x, dtype=np.float32)
    pe = np.asarray(pe, dtype=np.float32)
    content_mask = np.asarray(content_mask)
    Wqkv = np.asarray(Wqkv, dtype=np.float32)
    bqkv = np.asarray(bqkv, dtype=np.float32)
    Wc = np.asarray(Wc, dtype=np.float32)
    bc = np.asarray(bc, dtype=np.float32)

    idx = np.arange(S)
    causal = idx[None, :] > idx[:, None]
    if not np.array_equal(content_mask, np.broadcast_to(causal[:, :, None], (S, S, B))):
        return _numpy_fallback(x, pe, content_mask, Wqkv, bqkv, Wc, bc)

    from concourse.bass_utils import run_bass_kernel_spmd

    nc = _get_nc()
    in_maps = _make_in_maps(x, pe, Wqkv, bqkv, Wc)
    res = run_bass_kernel_spmd(nc, in_maps, core_ids=list(range(NCORE)))
    out = np.empty((S, B, C), np.float32)
    bc_eff = bc + Wc @ bqkv[2 * C:3 * C]   # v-bias folded through the output proj
    for b in range(B):
        acc = res.results[b * 4]["outp"].astype(np.float32).copy()
        for g in range(1, 4):
            acc += res.results[b * 4 + g]["outp"]
        out[:, b, :] = acc.T + bc_eff
    return out



# revision 8
# speedup vs baseline: 1.6607x; 1.6607x over previous
"""Trainium2 Bass kernel for nn_CompressedCausalAttention.

Sharding: 8 cores = 2 batches x 4 head-groups (2 heads each).
Host prep: xpe = (x+pe)^T in bf16 (cuts HBM reads ~3x and removes the
on-device add); weights transposed/sliced per core.

Per-core dataflow, window-pipelined (windows w = 512 s-columns):
  qkv(w):  q,k chan-major via PE (+bias via ACT); v seq-major, tight
           128-wide, staged to SBUF by DVE with a ones column appended
           (softmax denominator rides the AV matmul).
  attn(w): flash loop over 128-row t-chunks, scores in (t-part, s-free),
           staircase mask accumulated via PE (tri matmul), exp on ACT,
           AV accumulated in PSUM.
  tail(w): denominator row -> reciprocal_approx_fast (DVE, single pass)
           -> PE ones-broadcast -> DVE multiply.
  out(w):  partial out-projection, DVE copy to bf16, DMA out.
qkv(w+1) is emitted between attn(w) and tail(w) so the PE queue never
stalls on the DVE reciprocal chain.
Host: sums the 4 per-batch partials, adds bc (+ v-bias folded through Wc).
"""

import numpy as np
import ml_dtypes

S, B, C, H = 2048, 2, 512, 8
CC = C // H            # 64
HPC = 2                # heads per core
NCORE = 8
SW = 512               # s window (free dim of score tiles)
TCH = 128              # t chunk (partition dim of score tiles)
NW = S // SW           # 4 windows
TEMP = 1.0 / 8.0       # 1/sqrt(CC)
BIGNEG = -30000.0

_CACHE = {}


def _build_bass():
    import concourse.bass as bass
    import concourse.mybir as mybir
    import concourse.tile as tile
    from concourse import bacc

    f32 = mybir.dt.float32
    bf16 = mybir.dt.bfloat16

    nc = bacc.Bacc("TRN2", target_bir_lowering=False)
    xpet = nc.declare_dram_parameter("xpet", [NW, 4, 128, SW], bf16, isOutput=False)
    w3t = nc.declare_dram_parameter("w3t", [4, 128, 384], bf16, isOutput=False)
    b3 = nc.declare_dram_parameter("b3", [128, 2], f32, isOutput=False)
    wct = nc.declare_dram_parameter("wct", [128, C], bf16, isOutput=False)
    tri = nc.declare_dram_parameter("tri", [128, 128], bf16, isOutput=False)
    id128 = nc.declare_dram_parameter("id128", [128, 128], bf16, isOutput=False)
    outp = nc.declare_dram_parameter("outp", [C, S], bf16, isOutput=True)

    Ident = mybir.ActivationFunctionType.Identity
    Exp = mybir.ActivationFunctionType.Exp

    with tile.TileContext(nc) as tc:
        with (
            tc.tile_pool(name="singles", bufs=1) as singles,
            tc.tile_pool(name="xp", bufs=2) as xp,
            tc.tile_pool(name="pbp", bufs=3) as pbp,
            tc.tile_pool(name="atp", bufs=2) as atp,
            tc.tile_pool(name="rbp", bufs=2) as rbp,
            tc.tile_pool(name="osp", bufs=4) as osp,
            tc.tile_pool(name="scp", bufs=2, space="PSUM") as scp,
            tc.tile_pool(name="avp", bufs=1, space="PSUM") as avp,
            tc.tile_pool(name="smp", bufs=2, space="PSUM") as smp,
        ):
            # ---- constants ----
            tri_sb = singles.tile([128, 128], bf16, tag="tri")
            nc.sync.dma_start(out=tri_sb, in_=tri[:, :])
            id_sb = singles.tile([128, 128], bf16, tag="id128")
            nc.sync.dma_start(out=id_sb, in_=id128[:, :])
            w3t_sb = singles.tile([128, 4, 384], bf16, tag="w3t")
            for k in range(4):
                nc.sync.dma_start(out=w3t_sb[:, k, :], in_=w3t[k])
            b3_sb = singles.tile([128, 2], f32, tag="b3")
            nc.sync.dma_start(out=b3_sb, in_=b3[:, :])
            wct_sb = singles.tile([128, C], bf16, tag="wct")
            nc.sync.dma_start(out=wct_sb, in_=wct[:, :])

            qTs = [singles.tile([128, SW], bf16, tag=f"qT{w}", name=f"qT{w}") for w in range(NW)]
            kTs = [singles.tile([128, SW], bf16, tag=f"kT{w}", name=f"kT{w}") for w in range(NW)]
            # v seq-major per window: [t-part, chunk, head, 64 ch + ones col]
            vsb = [singles.tile([128, 4, HPC, CC + 1], bf16, tag=f"vsb{w}", name=f"vsb{w}")
                   for w in range(NW)]
            for w in range(NW):
                nc.vector.memset(vsb[w][:, :, :, CC:CC + 1], 1.0)

            def qkv(w):
                xw = xp.tile([128, 4, SW], bf16, tag="xpe")
                for k in range(4):
                    nc.sync.dma_start(out=xw[:, k, :], in_=xpet[w, k])
                for blk, dst in ((0, qTs[w]), (1, kTs[w])):
                    ps = smp.tile([128, SW], f32, tag="sm")
                    for k in range(4):
                        nc.tensor.matmul(
                            ps,
                            lhsT=w3t_sb[:, k, blk * 128:(blk + 1) * 128],
                            rhs=xw[:, k, :],
                            start=(k == 0), stop=(k == 3),
                        )
                    nc.scalar.activation(
                        out=dst, in_=ps, func=Ident,
                        bias=b3_sb[:, blk:blk + 1], scale=1.0,
                    )
                vps = smp.tile([128, 4, HPC, CC], f32, tag="sm")
                for tj in range(4):
                    for k in range(4):
                        nc.tensor.matmul(
                            vps[:, tj],
                            lhsT=xw[:, k, tj * 128:(tj + 1) * 128],
                            rhs=w3t_sb[:, k, 256:384],
                            start=(k == 0), stop=(k == 3),
                        )
                nc.vector.tensor_copy(out=vsb[w][:, :, :, 0:CC], in_=vps)

            qkv(0)
            for i in range(NW):
                # ---- attention window i ----
                jmax = 4 * i + 3
                avs = [avp.tile([CC + 1, SW], f32, tag=f"av{h}", name=f"av{h}_{i}") for h in range(HPC)]
                atn = atp.tile([128, SW], bf16, tag="atn")
                for j in range(jmax + 1):
                    # D = first unmasked column of this (t-chunk, s-window);
                    # columns [0:D) are strictly-future for every row.
                    D = max(0, TCH * j - SW * i)
                    wj, jj = j // 4, j % 4
                    sc = scp.tile([128, HPC, SW], f32, tag="sc")
                    for h in range(HPC):
                        nc.tensor.matmul(
                            sc[:, h, D:SW],
                            lhsT=kTs[wj][h * CC:(h + 1) * CC, jj * TCH:(jj + 1) * TCH],
                            rhs=qTs[i][h * CC:(h + 1) * CC, D:SW],
                            start=True, stop=True,
                        )
                        if j >= 4 * i:
                            # staircase mask: accumulate -30000*lower_tri via PE
                            nc.tensor.matmul(
                                sc[:, h, D:D + TCH], lhsT=tri_sb, rhs=id_sb,
                                start=False, stop=True, skip_group_check=True,
                            )
                    pb = pbp.tile([128, HPC, SW], bf16, tag="pb")
                    nc.scalar.activation(
                        out=pb[:, :, D:SW], in_=sc[:, :, D:SW], func=Exp, scale=TEMP,
                    )
                    for h in range(HPC):
                        nc.tensor.matmul(
                            avs[h][:, D:SW], lhsT=vsb[wj][:, jj, h, :],
                            rhs=pb[:, h, D:SW],
                            start=(j == 0), stop=(j == jmax),
                        )
                # next window's qkv goes on the PE queue ahead of the tail so
                # PE keeps busy while DVE computes the reciprocal
                if i + 1 < NW:
                    qkv(i + 1)
                # ---- softmax tail ----
                for h in range(HPC):
                    # den row must be staged to SBUF: reciprocal_approx_fast
                    # (custom DVE) reads garbage from PSUM on HW
                    dd = rbp.tile([1, SW], f32, tag="dd")
                    nc.vector.tensor_copy(out=dd, in_=avs[h][CC:CC + 1, :])
                    r1 = rbp.tile([1, SW], f32, tag="r1")
                    nc.vector.reciprocal_approx_fast(out=r1, in_=dd)
                    rb = rbp.tile([CC, SW], f32, tag="rb")
                    nc.gpsimd.partition_broadcast(out_ap=rb, in_ap=r1, channels=CC)
                    nc.vector.tensor_mul(
                        out=atn[h * CC:(h + 1) * CC, :],
                        in0=avs[h][0:CC, :], in1=rb,
                    )
                # ---- partial out-projection for this window ----
                for d in range(4):
                    op = smp.tile([128, SW], f32, tag="sm")
                    nc.tensor.matmul(
                        op, lhsT=wct_sb[:, d * 128:(d + 1) * 128],
                        rhs=atn,
                        start=True, stop=True,
                    )
                    ob = osp.tile([128, SW], bf16, tag="ob")
                    nc.vector.tensor_copy(out=ob, in_=op)
                    nc.sync.dma_start(
                        out=outp[d * 128:(d + 1) * 128, i * SW:(i + 1) * SW], in_=ob
                    )

    nc.compile()
    return nc


def _get_nc():
    if "nc" not in _CACHE:
        _CACHE["nc"] = _build_bass()
    return _CACHE["nc"]


def _make_in_maps(x, pe, Wqkv, bqkv, Wc):
    bf = ml_dtypes.bfloat16
    tt = np.arange(128)[:, None]
    kk = np.arange(128)[None, :]
    tri = np.where(kk < tt, np.float32(BIGNEG), np.float32(0.0)).astype(bf).T.copy()
    id128 = np.eye(128, dtype=np.float32).astype(bf)

    xpet_b = {}
    for b in range(B):
        t = (x[:, b, :] + pe[:, b, :]).T.astype(bf).reshape(4, 128, NW, SW)
        xpet_b[b] = np.ascontiguousarray(t.transpose(2, 0, 1, 3))

    in_maps = []
    for core in range(NCORE):
        b, hg = core // 4, core % 4
        lo = hg * 128
        W3 = np.concatenate([Wqkv[lo:lo + 128], Wqkv[C + lo:C + lo + 128],
                             Wqkv[2 * C + lo:2 * C + lo + 128]])
        w3t = np.ascontiguousarray(W3.T).reshape(4, 128, 384).astype(bf)
        b3 = np.stack([bqkv[lo:lo + 128], bqkv[C + lo:C + lo + 128]], axis=1)
        b3 = np.ascontiguousarray(b3).astype(np.float32)
        wct = np.ascontiguousarray(Wc[:, lo:lo + 128].T).astype(bf)
        in_maps.append({
            "xpet": xpet_b[b], "w3t": w3t, "b3": b3,
            "wct": wct, "tri": tri, "id128": id128,
        })
    return in_maps


def _numpy_fallback(x, pe, content_mask, Wqkv, bqkv, Wc, bc):
    xpe = (x + pe).astype(np.float32)
    qkv = xpe.reshape(-1, C) @ Wqkv.T + bqkv
    qkv = qkv.reshape(S, B, 3 * C)
    q, k, v = np.split(qkv, 3, axis=-1)
    q = q.reshape(S, B, H, CC)
    k = k.reshape(S, B, H, CC)
    v = v.reshape(S, B, H, CC)
    out = np.empty((S, B, C), np.float32)
    for b in range(B):
        for h in range(H):
            sc = (q[:, b, h] @ k[:, b, h].T) * np.float32(TEMP)
            sc = np.where(content_mask[:, :, b], -np.inf, sc)
            sc = sc - sc.max(axis=1, keepdims=True)
            p = np.exp(sc)
            p /= p.sum(axis=1, keepdims=True)
            out[:, b, h * CC:(h + 1) * CC] = p @ v[:, b, h]
    return (out.reshape(-1, C) @ Wc.T + bc).reshape(S, B, C).astype(np.float32)


def kernel(x, pe, content_mask, pad, Wqkv, bqkv, Wc, bc):
    x = np.asarray(x, dtype=np.float32)
    pe = np.asarray(pe, dtype=np.float32)
    content_mask = np.asarray(content_mask)
    Wqkv = np.asarray(Wqkv, dtype=np.float32)
    bqkv = np.asarray(bqkv, dtype=np.float32)
    Wc = np.asarray(Wc, dtype=np.float32)
    bc = np.asarray(bc, dtype=np.float32)

    idx = np.arange(S)
    causal = idx[None, :] > idx[:, None]
    if not np.array_equal(content_mask, np.broadcast_to(causal[:, :, None], (S, S, B))):
        return _numpy_fallback(x, pe, content_mask, Wqkv, bqkv, Wc, bc)

    from concourse.bass_utils import run_bass_kernel_spmd

    nc = _get_nc()
    in_maps = _make_in_maps(x, pe, Wqkv, bqkv, Wc)
    res = run_bass_kernel_spmd(nc, in_maps, core_ids=list(range(NCORE)))
    out = np.empty((S, B, C), np.float32)
    bc_eff = bc + Wc @ bqkv[2 * C:3 * C]   # v-bias folded through the output proj
    for b in range(B):
        acc = res.results[b * 4]["outp"].astype(np.float32).copy()
        for g in range(1, 4):
            acc += res.results[b * 4 + g]["outp"]
        out[:, b, :] = acc.T + bc_eff
    return out
